# revision 49
# baseline (speedup 1.0000x reference)
"""Trainium2 Bass kernel for ExtensibleAttention (sparse_attention).

Strategy: data-parallel over the 65536 tokens (N*L flattened) across 8
NeuronCores; the small 256-dim projection weights are replicated. All
per-token math is fused into one pass per 512-token tile:

  q/k/v/pos projections as PE matmuls in [C, T] layout (channel on
  partitions, token on free dim), with q+pos / k+pos fused into the PSUM
  accumulation; offset MLP (relu + second projection) likewise; the
  grid-sample weight w, softmax over K=4 sample points, and the final
  out-projection all on-chip.

Inputs are pre-transposed to [C, T] on the host (numpy) so the kernel
needs no on-chip transposes: matmul contracts over the partition dim, so
activations must be channel-major anyway. Head reductions (sum over d
within a head), the k-broadcast of qk, the sum over K, and the
head->channel broadcast of wv are done as matmuls against small constant
0/1 matrices. The Wo2 columns are host-permuted from (h,k,c) to (c,h,k)
order so the x/y coordinates occupy partition halves, making the
grid-sample weight product a single partition-offset vector multiply.
"""

import numpy as np
from contextlib import ExitStack

import concourse.bacc as bacc
import concourse.tile as tile
from concourse import mybir

F32 = mybir.dt.float32
F32R = mybir.dt.float32r
AF = mybir.ActivationFunctionType

N, L, C, H, KP, D = 4, 16384, 256, 8, 4, 32
NCORES = 8
TOKS = N * L // NCORES  # 8192 tokens per core
TLOAD = 512             # tokens per DMA load tile
TCOMP = 512             # tokens per compute tile (PSUM free-dim limit, fp32)
SIGMA = float(1.0 / np.sqrt(D))


def _build(toks=TOKS, tload=TLOAD, with_bias=False):
    nc = bacc.Bacc(trn_type="TRN2")
    dram = {}

    def din(name, shape, dt=None):
        dram[name] = nc.dram_tensor(name, list(shape), dt or F32R,
                                    kind="ExternalInput")
        return dram[name]

    xq = din("xq", (128, 2, toks))
    xk = din("xk", (128, 2, toks))
    xv = din("xv", (128, 2, toks))
    xp = din("xp", (128, 2, toks))
    ref = din("ref", (2, toks))
    din("wq", (128, 2, 256))
    din("wk", (128, 2, 256))
    din("wv", (128, 2, 256))
    din("wp", (128, 2, 256))
    din("wo1", (128, 2, 512))
    din("wo2", (128, 4, 64))
    din("wo", (128, 2, 256))
    din("bo1", (128, 4))
    din("bwof", (64, 1))
    din("smat", (64, 32))
    din("amat", (128, 64))
    din("cmat", (32, 8))
    din("bmat", (8, 256))
    din("pmat", (2, 64))
    if with_bias:
        din("ones", (1, 512))
        din("bqp", (1, 256))
        din("bkp", (1, 256))
        din("bvr", (1, 256))
        din("bor", (1, 256))
    out = nc.dram_tensor("out", [toks, 256], F32, kind="ExternalOutput")

    nload = toks // tload
    nsub = tload // TCOMP
    T = TCOMP

    with tile.TileContext(nc) as tc, ExitStack() as ctx:
        singles = ctx.enter_context(tc.tile_pool(name="singles", bufs=1))
        inp = ctx.enter_context(tc.tile_pool(name="inp", bufs=4))
        work = ctx.enter_context(tc.tile_pool(name="work", bufs=2))
        psA = ctx.enter_context(tc.tile_pool(name="psA", bufs=3, space="PSUM"))
        psB = ctx.enter_context(tc.tile_pool(name="psB", bufs=5, space="PSUM"))

        def load1(name, shape, dt=F32R):
            t = singles.tile(list(shape), dt, name=f"sb_{name}")
            nc.sync.dma_start(out=t, in_=dram[name][:])
            return t

        mm = nc.tensor.matmul

        def load_tile(lt):
            t0 = lt * tload
            xv_t = inp.tile([128, 2, tload], F32R, tag="xv")
            nc.sync.dma_start(out=xv_t, in_=xv[:, :, t0:t0 + tload])
            xq_t = inp.tile([128, 2, tload], F32R, tag="xq")
            nc.sync.dma_start(out=xq_t, in_=xq[:, :, t0:t0 + tload])
            xp_t = inp.tile([128, 2, tload], F32R, tag="xp")
            nc.sync.dma_start(out=xp_t, in_=xp[:, :, t0:t0 + tload])
            xk_t = inp.tile([128, 2, tload], F32R, tag="xk")
            nc.sync.dma_start(out=xk_t, in_=xk[:, :, t0:t0 + tload])
            ref_t = inp.tile([2, tload], F32R, tag="ref")
            nc.sync.dma_start(out=ref_t, in_=ref[:, t0:t0 + tload])
            return xq_t, xp_t, xk_t, xv_t, ref_t

        def stage1(ld, lo, tz):
            """Projection matmuls + q*k product + hidden/offset MLP."""
            xq_t, xp_t, xk_t, xv_t, ref_t = ld
            s = slice(lo, lo + tz)

            # v = value@Wv  (per-chunk 1-bank PSUM tiles: slot reuse only
            # depends on ACT copies of the previous tile, never on DVE)
            v_sb = work.tile([128, 2, tz], F32, tag="v", bufs=3)
            for mc in range(2):
                m128 = slice(mc * 128, (mc + 1) * 128)
                v_ps = psA.tile([128, tz], F32, tag="bigA")
                mm(v_ps, wv_s[:, 0, m128], xv_t[:, 0, s], start=True, stop=False)
                mm(v_ps, wv_s[:, 1, m128], xv_t[:, 1, s], start=False,
                   stop=not with_bias)
                if with_bias:
                    mm(v_ps, bvr_s[:, m128], ones_s[:, :tz], start=False, stop=True)
                nc.scalar.copy(v_sb[:, mc, :], v_ps)

            # q/k projections (+pos fused into the PSUM accumulation) and the
            # q*k product, one 128-channel chunk at a time so each chunk's
            # PSUM bank frees while the next chunk's matmuls run
            q_sb = work.tile([128, 2, tz], F32, tag="qsb", bufs=1)
            k_sb = work.tile([128, 2, tz], F32, tag="ksb", bufs=1)
            m_sb = work.tile([128, 2, tz], F32R, tag="m", bufs=2)
            for mc in range(2):
                m128 = slice(mc * 128, (mc + 1) * 128)
                q_ps = psA.tile([128, tz], F32, tag="bigA")
                mm(q_ps, wq_s[:, 0, m128], xq_t[:, 0, s], start=True, stop=False)
                mm(q_ps, wq_s[:, 1, m128], xq_t[:, 1, s], start=False, stop=False)
                mm(q_ps, wp_s[:, 0, m128], xp_t[:, 0, s], start=False, stop=False)
                mm(q_ps, wp_s[:, 1, m128], xp_t[:, 1, s], start=False,
                   stop=not with_bias)
                if with_bias:
                    mm(q_ps, bqp_s[:, m128], ones_s[:, :tz], start=False, stop=True)
                k_ps = psA.tile([128, tz], F32, tag="bigA")
                mm(k_ps, wk_s[:, 0, m128], xk_t[:, 0, s], start=True, stop=False)
                mm(k_ps, wk_s[:, 1, m128], xk_t[:, 1, s], start=False, stop=False)
                mm(k_ps, wp_s[:, 0, m128], xp_t[:, 0, s], start=False, stop=False)
                mm(k_ps, wp_s[:, 1, m128], xp_t[:, 1, s], start=False,
                   stop=not with_bias)
                if with_bias:
                    mm(k_ps, bkp_s[:, m128], ones_s[:, :tz], start=False, stop=True)
                # ACT copies release the PSUM banks immediately; the q*k
                # product runs on the otherwise-idle GPSIMD (SBUF-only)
                nc.scalar.copy(q_sb[:, mc, :], q_ps)
                nc.scalar.copy(k_sb[:, mc, :], k_ps)
                nc.gpsimd.tensor_mul(m_sb[:, mc, :], q_sb[:, mc, :],
                                     k_sb[:, mc, :])
            # hidden = relu(query@Wo1 + bo1), 4 chunks of 128
            hid_sb = work.tile([128, 4, tz], F32R, tag="hid", bufs=1)
            for j in range(4):
                h_ps = psB.tile([128, tz], F32, tag="small")
                j128 = slice(j * 128, (j + 1) * 128)
                mm(h_ps, wo1_s[:, 0, j128], xq_t[:, 0, s], start=True, stop=False)
                mm(h_ps, wo1_s[:, 1, j128], xq_t[:, 1, s], start=False, stop=True)
                nc.scalar.activation(hid_sb[:, j, :], h_ps, AF.Relu,
                                     bias=bo1_s[:, j:j + 1], scale=1.0)

            # off = hidden@Wo2p + ref, rows = (c,h,k) with x coords in
            # partitions 0-31 and y coords in 32-63
            off_ps = psB.tile([64, tz], F32, tag="small")
            for j in range(4):
                mm(off_ps, wo2_s[:, j, :], hid_sb[:, j, :],
                   start=(j == 0), stop=False)
            mm(off_ps, pmat_s, ref_t[:, s], start=False, stop=True)
            return m_sb, v_sb, off_ps, tz

        def stage2a(state):
            """Head-sum of q*k, grid-sample weight w, softmax partial sums."""
            m_sb, v_sb, off_ps, tz = state

            # qk head-sum one pipeline step after the GPSIMD q*k product so
            # the PE never waits on it
            qk_ps = psB.tile([32, tz], F32, tag="small")
            mm(qk_ps, amat_s[:, 0:32], m_sb[:, 0, :], start=True, stop=False)
            mm(qk_ps, amat_s[:, 32:64], m_sb[:, 1, :], start=False, stop=True)
            qk_sb = work.tile([32, tz], F32, tag="qks")
            nc.vector.tensor_copy(qk_sb, qk_ps)

            # w = relu(1-|sp_x-.5|)*relu(1-|sp_y-.5|); the y half is moved
            # to partitions 0-31 with a PE row-select matmul since DVE can't
            # pair operands at different base partitions
            t1_sb = work.tile([64, tz], F32, tag="t1")
            nc.scalar.activation(t1_sb, off_ps, AF.Abs, bias=bwof_s, scale=1.0)
            t2_sb = work.tile([64, tz], F32R, tag="t2")
            nc.scalar.activation(t2_sb, t1_sb, AF.Relu, bias=1.0, scale=-1.0)
            t2y_ps = psB.tile([32, tz], F32, tag="small")
            mm(t2y_ps, smat_s, t2_sb, start=True, stop=True)
            w_sb = work.tile([32, tz], F32, tag="w")
            nc.vector.tensor_mul(w_sb, t2_sb[0:32, :], t2y_ps)

            # softmax over K: e = exp(qk*w/sqrt(D))
            lg_sb = work.tile([32, tz], F32, tag="lg")
            nc.vector.tensor_mul(lg_sb, qk_sb, w_sb)
            e_sb = work.tile([32, tz], F32R, tag="e")
            nc.scalar.activation(e_sb, lg_sb, AF.Exp, bias=0.0, scale=SIGMA)
            ew_sb = work.tile([32, tz], F32R, tag="ew")
            nc.vector.tensor_mul(ew_sb, e_sb, w_sb)
            s1_ps = psB.tile([8, tz], F32, tag="small")
            mm(s1_ps, cmat_s, e_sb, start=True, stop=True)
            s2_ps = psB.tile([8, tz], F32, tag="small")
            mm(s2_ps, cmat_s, ew_sb, start=True, stop=True)
            return s1_ps, s2_ps, v_sb, tz

        def stage2b(state, g0):
            """Softmax normalization, ov = v*wv, out-projection, store."""
            s1_ps, s2_ps, v_sb, tz = state
            r1_sb = work.tile([8, tz], F32, tag="r1")
            nc.vector.reciprocal(r1_sb, s1_ps)
            wv_sb = work.tile([8, tz], F32R, tag="wvv")
            nc.vector.tensor_mul(wv_sb, s2_ps, r1_sb)

            # ov = v * wv (broadcast head->channels via matmul)
            ov_sb = work.tile([128, 2, tz], F32R, tag="ov")
            for mc in range(2):
                wvx_ps = psB.tile([128, tz], F32, tag="small")
                mm(wvx_ps, bmat_s[:, mc * 128:(mc + 1) * 128], wv_sb,
                   start=True, stop=True)
                nc.vector.tensor_mul(ov_sb[:, mc, :], v_sb[:, mc, :], wvx_ps)

            # out = ov.T @ Wout (+bout), token-major [T, 256]
            o_sb = work.tile([128, tz // 128, 256], F32, tag="osb")
            for q4 in range(tz // 128):
                o_ps = psB.tile([128, 256], F32, tag="small")
                q128 = slice(q4 * 128, (q4 + 1) * 128)
                mm(o_ps, ov_sb[:, 0, q128], wo_s[:, 0, :], start=True, stop=False)
                mm(o_ps, ov_sb[:, 1, q128], wo_s[:, 1, :], start=False,
                   stop=not with_bias)
                if with_bias:
                    mm(o_ps, ones_s[:, 0:128], bor_s, start=False, stop=True)
                nc.vector.tensor_copy(o_sb[:, q4, :], o_ps)
            nc.sync.dma_start(
                out=out[g0:g0 + tz, :].rearrange("(s2 p) c -> p s2 c", p=128),
                in_=o_sb)

        # 3-deep software pipeline: per iteration emit tile i's matmul-heavy
        # stage1, then tile i-2's output tail (stage2b), then tile i-1's
        # softmax chain (stage2a) — PE stays dense while ACT/DVE chains of
        # earlier tiles drain. stage2b(i-2) must precede stage2a(i-1) so the
        # s1/s2 PSUM slots recycle in trace order.
        assert nsub == 1
        # one full-width work unit per load tile (half-tile drain splitting
        # measured net-worse in the cost model: per-op overheads exceed the
        # drain savings)
        units = [(lt, 0, tload) for lt in range(nload)]
        p1 = p2 = None  # (state, g0) for stage2a / stage2b
        # first input tile before the weights so the PE can start ASAP;
        # weights ordered by first use
        wv_s = load1("wv", (128, 2, 256))
        ld = load_tile(0)
        wq_s = load1("wq", (128, 2, 256))
        wp_s = load1("wp", (128, 2, 256))
        wk_s = load1("wk", (128, 2, 256))
        wo1_s = load1("wo1", (128, 2, 512))
        bo1_s = load1("bo1", (128, 4))
        amat_s = load1("amat", (128, 64))
        wo2_s = load1("wo2", (128, 4, 64))
        pmat_s = load1("pmat", (2, 64))
        bwof_s = load1("bwof", (64, 1))
        smat_s = load1("smat", (64, 32))
        cmat_s = load1("cmat", (32, 8))
        bmat_s = load1("bmat", (8, 256))
        wo_s = load1("wo", (128, 2, 256))
        if with_bias:
            bqp_s = load1("bqp", (1, 256))
            bkp_s = load1("bkp", (1, 256))
            bvr_s = load1("bvr", (1, 256))
            bor_s = load1("bor", (1, 256))
            ones_s = load1("ones", (1, 512))
        ld_next = None
        cur_lt = 0
        for ui, (lt, lo, tz) in enumerate(units):
            if ui + 1 < len(units) and units[ui + 1][0] != lt:
                ld_next = load_tile(units[ui + 1][0])
            state = stage1(ld, lo, tz)
            if p2 is not None:
                stage2b(*p2)
                p2 = None
            if p1 is not None:
                st2, g0p = p1
                p2 = (stage2a(st2), g0p)
            p1 = (state, lt * tload + lo)
            if ui + 1 < len(units) and units[ui + 1][0] != lt:
                ld = ld_next
        if p2 is not None:
            stage2b(*p2)
        st2, g0p = p1
        stage2b(stage2a(st2), g0p)

    nc.compile()
    return nc


def _consts():
    amat = np.zeros((128, 64), np.float32)
    for mc in range(2):
        for d in range(128):
            h = mc * 4 + d // 32
            for k in range(KP):
                amat[d, mc * 32 + h * KP + k] = 1.0
    cmat = np.zeros((32, 8), np.float32)
    for j in range(32):
        cmat[j, j // KP] = 1.0
    bmat = np.zeros((8, 256), np.float32)
    for mc in range(2):
        for c in range(128):
            bmat[mc * 4 + c // 32, mc * 128 + c] = 1.0
    pmat = np.zeros((2, 64), np.float32)
    for r in range(64):
        pmat[r // 32, r] = 1.0
    smat = np.zeros((64, 32), np.float32)
    for j in range(32):
        smat[32 + j, j] = 1.0
    return amat, cmat, bmat, pmat, smat


def _wsplit(w):
    # [256, O] -> [128, 2, O]  (row kc*128+p  ->  [p, kc, :])
    o = w.shape[1]
    return np.ascontiguousarray(w.reshape(2, 128, o).transpose(1, 0, 2))


def _xsplit(x):
    # [T, 256] token-major -> [128, 2, T] channel-major chunks
    t = x.shape[0]
    return np.ascontiguousarray(x.T.reshape(2, 128, t).transpose(1, 0, 2))


def _host_maps(inputs, toks, ncores):
    f32 = lambda v: np.asarray(v, dtype=np.float32)
    query = f32(inputs["query"]).reshape(-1, C)
    key = f32(inputs["key"]).reshape(-1, C)
    value = f32(inputs["value"]).reshape(-1, C)
    pos = f32(inputs["pos_embed"]).reshape(-1, C)
    refp = f32(inputs["reference_points"]).reshape(-1, 2)

    # permute Wo2 columns (h,k,c) -> (c,h,k)
    perm = [h * (KP * 2) + k * 2 + c for c in range(2) for h in range(H)
            for k in range(KP)]
    wo2p = f32(inputs["Wo2"])[:, perm]
    bo2p = f32(inputs["bo2"])[perm]

    amat, cmat, bmat, pmat, smat = _consts()
    bqp = f32(inputs["bq"]) + f32(inputs["bpos"])
    bkp = f32(inputs["bk"]) + f32(inputs["bpos"])
    bv = f32(inputs["bv"])
    bout = f32(inputs["bout"])
    with_bias = any(np.any(b != 0) for b in (bqp, bkp, bv, bout))

    wo2r = np.ascontiguousarray(wo2p.reshape(4, 128, 64).transpose(1, 0, 2))
    shared = {
        "wq": _wsplit(f32(inputs["Wq"])),
        "wk": _wsplit(f32(inputs["Wk"])),
        "wv": _wsplit(f32(inputs["Wv"])),
        "wp": _wsplit(f32(inputs["Wpos"])),
        "wo1": _wsplit(f32(inputs["Wo1"])),
        "wo2": wo2r,
        "wo": _wsplit(f32(inputs["Wout"])),
        "bo1": np.ascontiguousarray(f32(inputs["bo1"]).reshape(4, 128).T),
        "bwof": np.ascontiguousarray((bo2p - 0.5).reshape(64, 1)),
        "smat": smat,
        "amat": amat, "cmat": cmat, "bmat": bmat, "pmat": pmat,
    }
    if with_bias:
        shared["ones"] = np.ones((1, 512), np.float32)
        shared["bqp"] = bqp.reshape(1, 256)
        shared["bkp"] = bkp.reshape(1, 256)
        shared["bvr"] = bv.reshape(1, 256)
        shared["bor"] = bout.reshape(1, 256)

    in_maps = []
    for cid in range(ncores):
        sl = slice(cid * toks, (cid + 1) * toks)
        m = dict(shared)
        m["xq"] = _xsplit(query[sl])
        m["xk"] = _xsplit(key[sl])
        m["xv"] = _xsplit(value[sl])
        m["xp"] = _xsplit(pos[sl])
        m["ref"] = np.ascontiguousarray(refp[sl].T)
        in_maps.append(m)
    return in_maps, with_bias


_NC_CACHE = {}


def kernel(**inputs):
    from concourse.bass_utils import run_bass_kernel_spmd

    in_maps, with_bias = _host_maps(inputs, TOKS, NCORES)
    ck = ("full", with_bias)
    if ck not in _NC_CACHE:
        _NC_CACHE[ck] = _build(toks=TOKS, tload=TLOAD, with_bias=with_bias)
    nc = _NC_CACHE[ck]
    res = run_bass_kernel_spmd(nc, in_maps, core_ids=list(range(NCORES)))
    outs = [r["out"] for r in res.results]
    full = np.concatenate(outs, axis=0).reshape(N, L, C)
    return np.ascontiguousarray(full.astype(np.float32))
